# revision 1
# baseline (speedup 1.0000x reference)
"""AugmentedLstm Trainium2 kernel.

Math (faithful to the reference module):
    g_t  = px_t + (h_{t-1} @ W + b)         with px_t = x_t @ W + b
         = (x_t + h_{t-1}) @ W + 2b         (same W projects input and state!)
    i,f  = sigmoid(g[0:512]), sigmoid(g[512:1024])
    m    = tanh(g[1024:1536])
    o    = sigmoid(g[1536:2048]);  hw = sigmoid(g[2048:2560])
    c_t  = i*m + f*c_{t-1}
    out  = o * tanh(c_t)
    h_t  = hw*out + (1-hw)*px5,   px5 = x_t @ W[:,2560:3072] + b[2560:3072]
    h_t  = mask(t < len) * h_t    (sorted-desc ragged lengths)

The 6th gate block of the recurrent matmul is never used (highway reads raw
px5), so the recurrent matmul only streams 5*H columns.  Batch rows are
independent through the recurrence, so retired rows may compute garbage as
long as stores are masked — c needs no masking at all.

Sharding: data-parallel over batch, 16 rows per core, full local scan.
Layout: batch-partition ([16, cols]); h is PE-transposed each step to form
the stationary operand u^T = (x_t + h)^T; x^T arrives pre-transposed from
the host, so px5's matmul reuses it directly.
"""

import numpy as np
from contextlib import ExitStack

import concourse.bass as bass
import concourse.bacc as bacc
import concourse.tile as tile
import concourse.mybir as mybir
from concourse.bass_utils import run_bass_kernel_spmd

F32 = mybir.dt.float32
AF = mybir.ActivationFunctionType
ALU = mybir.AluOpType

B, T, H = 128, 512, 512
NCORES = 8
BSH = B // NCORES          # 16 rows per core
G5 = 5 * H                 # 2560 columns for the 5 used gates
KC = H // 128              # 4 contraction chunks


def build_nc(t_steps=T, bsh=BSH, variant=1):
    if variant == 2:
        return build_nc_v2(t_steps, bsh)
    if variant == 3:
        return build_nc_v3(t_steps, bsh)
    if variant == 4:
        return build_nc_v4(t_steps, bsh)
    nc = bacc.Bacc(
        "TRN2",
        target_bir_lowering=False,
        debug=False,
        enable_asserts=False,
        num_devices=NCORES,
    )
    xT_d = nc.dram_tensor("xT", [t_steps, H, bsh], F32, kind="ExternalInput")
    mask_d = nc.dram_tensor("maskT", [bsh, t_steps], F32, kind="ExternalInput")
    w5_d = nc.dram_tensor("w5", [128, KC, G5], F32, kind="ExternalInput")
    w6_d = nc.dram_tensor("w6", [128, KC, H], F32, kind="ExternalInput")
    b5_d = nc.dram_tensor("b5", [1, G5], F32, kind="ExternalInput")
    b6_d = nc.dram_tensor("b6", [1, H], F32, kind="ExternalInput")
    id_d = nc.dram_tensor("ident", [bsh, bsh], F32, kind="ExternalInput")
    out_d = nc.dram_tensor("out", [bsh, t_steps, H], F32, kind="ExternalOutput")

    with tile.TileContext(nc) as tc:
        with ExitStack() as ctx:
            const = ctx.enter_context(tc.tile_pool(name="const", bufs=1))
            xpool = ctx.enter_context(tc.tile_pool(name="xp", bufs=4))
            upool = ctx.enter_context(tc.tile_pool(name="up", bufs=2))
            hpool = ctx.enter_context(tc.tile_pool(name="hp", bufs=2))
            cpool = ctx.enter_context(tc.tile_pool(name="cp", bufs=2))
            spool = ctx.enter_context(tc.tile_pool(name="sp", bufs=2))
            gpsum = ctx.enter_context(
                tc.tile_pool(name="gps", bufs=1, space=bass.MemorySpace.PSUM)
            )
            ppsum = ctx.enter_context(
                tc.tile_pool(name="pps", bufs=1, space=bass.MemorySpace.PSUM)
            )
            tpsum = ctx.enter_context(
                tc.tile_pool(name="tps", bufs=1, space=bass.MemorySpace.PSUM)
            )

            w5sb = const.tile([128, KC, G5], F32, tag="w5")
            nc.sync.dma_start(w5sb[:], w5_d[:])
            w6sb = const.tile([128, KC, H], F32, tag="w6")
            nc.sync.dma_start(w6sb[:], w6_d[:])
            b5sb = const.tile([1, G5], F32, tag="b5")
            nc.sync.dma_start(b5sb[:], b5_d[:])
            b6sb = const.tile([1, H], F32, tag="b6")
            nc.sync.dma_start(b6sb[:], b6_d[:])
            idsb = const.tile([bsh, bsh], F32, tag="id")
            nc.sync.dma_start(idsb[:], id_d[:])
            masksb = const.tile([bsh, t_steps], F32, tag="mask")
            nc.sync.dma_start(masksb[:], mask_d[:])
            ones1 = const.tile([1, bsh], F32, tag="ones")
            nc.vector.memset(ones1[:], 1.0)

            ht = hpool.tile([bsh, H], F32, tag="h")
            nc.vector.memset(ht[:], 0.0)
            ct = cpool.tile([bsh, H], F32, tag="c")
            nc.vector.memset(ct[:], 0.0)

            for t in range(t_steps):
                xt = xpool.tile([128, KC, bsh], F32, tag="xt")
                nc.sync.dma_start(
                    xt[:], xT_d[t].rearrange("(k p) b -> p k b", p=128)
                )

                # u^T = h^T + x^T  (stationary operand, [128, KC, bsh])
                ptr = tpsum.tile([128, KC * bsh], F32, tag="ptr")
                for k in range(KC):
                    nc.tensor.transpose(
                        ptr[:, k * bsh : (k + 1) * bsh],
                        ht[:, k * 128 : (k + 1) * 128],
                        idsb[:],
                    )
                uT = upool.tile([128, KC, bsh], F32, tag="uT")
                for k in range(KC):
                    nc.vector.scalar_tensor_tensor(
                        uT[:, k, :],
                        ptr[:, k * bsh : (k + 1) * bsh],
                        1.0,
                        xt[:, k, :],
                        op0=ALU.mult,
                        op1=ALU.add,
                    )

                # g5 = (x+h) @ W5 + 2b  -> PSUM [bsh, 2560]
                g5 = gpsum.tile([bsh, G5], F32, tag="g5")
                for n in range(5):
                    gb = g5[:, n * 512 : (n + 1) * 512]
                    for k in range(KC):
                        nc.tensor.matmul(
                            gb,
                            uT[:, k, :],
                            w5sb[:, k, n * 512 : (n + 1) * 512],
                            start=(k == 0),
                            stop=False,
                        )
                    nc.tensor.matmul(
                        gb,
                        ones1[:],
                        b5sb[:, n * 512 : (n + 1) * 512],
                        start=False,
                        stop=True,
                    )

                # px5 = x @ W6 + b6 -> PSUM [bsh, 512]
                p6 = ppsum.tile([bsh, H], F32, tag="p6")
                for k in range(KC):
                    nc.tensor.matmul(
                        p6[:], xt[:, k, :], w6sb[:, k, :],
                        start=(k == 0), stop=False,
                    )
                nc.tensor.matmul(p6[:], ones1[:], b6sb[:], start=False, stop=True)

                # gates
                gs = spool.tile([bsh, 4 * H], F32, tag="gs")  # sig(i,f,o,hw)
                nc.scalar.activation(gs[:], g5[:, 0 : 4 * H], AF.Sigmoid)
                ms = spool.tile([bsh, H], F32, tag="ms")
                nc.scalar.activation(ms[:], g5[:, 4 * H : 5 * H], AF.Tanh)
                px5 = spool.tile([bsh, H], F32, tag="px5")
                nc.scalar.copy(px5[:], p6[:])

                im = spool.tile([bsh, H], F32, tag="im")
                nc.vector.tensor_mul(im[:], gs[:, 0:512], ms[:])
                fc = spool.tile([bsh, H], F32, tag="fc")
                nc.vector.tensor_mul(fc[:], gs[:, 512:1024], ct[:])
                cn = cpool.tile([bsh, H], F32, tag="c")
                nc.vector.tensor_add(cn[:], im[:], fc[:])
                tch = spool.tile([bsh, H], F32, tag="tch")
                nc.scalar.activation(tch[:], cn[:], AF.Tanh)
                h1 = spool.tile([bsh, H], F32, tag="h1")
                nc.vector.tensor_mul(h1[:], gs[:, 1024:1536], tch[:])
                d = spool.tile([bsh, H], F32, tag="d")
                nc.vector.tensor_sub(d[:], h1[:], px5[:])
                e = spool.tile([bsh, H], F32, tag="e")
                nc.vector.tensor_mul(e[:], gs[:, 1536:2048], d[:])
                hn = spool.tile([bsh, H], F32, tag="hn")
                nc.vector.tensor_add(hn[:], e[:], px5[:])
                hf = hpool.tile([bsh, H], F32, tag="h")
                nc.vector.tensor_scalar_mul(hf[:], hn[:], masksb[:, t : t + 1])

                nc.sync.dma_start(out_d[:, t, :], hf[:])

                ht = hf
                ct = cn

    nc.compile()
    return nc


def build_nc_v2(t_steps=T, bsh=BSH):
    """Col-tiled variant: the M=16 matmuls for gates i,f,o,hw run
    concurrently in 4 PE column-groups (tile_position=(0,32j)), landing at
    partition offsets 0/32/64/96 of ONE psum bank; m and px5 share a second
    bank at offsets 0/32.  One sigmoid ACT op covers all four sigma-gates
    ([112,512] — ACT cost is free-dim only), and [i;f] (x) [m;c_prev] packs
    into a single DVE op via co-locating m and c in one [48,512] tile."""
    nc = bacc.Bacc(
        "TRN2",
        target_bir_lowering=False,
        debug=False,
        enable_asserts=False,
        num_devices=NCORES,
    )
    xT_d = nc.dram_tensor("xT", [t_steps, H, bsh], F32, kind="ExternalInput")
    mask_d = nc.dram_tensor("maskT", [bsh, t_steps], F32, kind="ExternalInput")
    w5_d = nc.dram_tensor("w5", [128, KC, G5], F32, kind="ExternalInput")
    w6_d = nc.dram_tensor("w6", [128, KC, H], F32, kind="ExternalInput")
    b5_d = nc.dram_tensor("b5", [1, G5], F32, kind="ExternalInput")
    b6_d = nc.dram_tensor("b6", [1, H], F32, kind="ExternalInput")
    id_d = nc.dram_tensor("ident", [bsh, bsh], F32, kind="ExternalInput")
    out_d = nc.dram_tensor("out", [bsh, t_steps, H], F32, kind="ExternalOutput")

    with tile.TileContext(nc) as tc:
        with ExitStack() as ctx:
            const = ctx.enter_context(tc.tile_pool(name="const", bufs=1))
            xpool = ctx.enter_context(tc.tile_pool(name="xp", bufs=4))
            upool = ctx.enter_context(tc.tile_pool(name="up", bufs=2))
            hpool = ctx.enter_context(tc.tile_pool(name="hp", bufs=2))
            mcpool = ctx.enter_context(tc.tile_pool(name="mcp", bufs=2))
            spool = ctx.enter_context(tc.tile_pool(name="sp", bufs=2))
            gpsA = ctx.enter_context(
                tc.tile_pool(name="gpsA", bufs=2, space=bass.MemorySpace.PSUM)
            )
            gpsB = ctx.enter_context(
                tc.tile_pool(name="gpsB", bufs=2, space=bass.MemorySpace.PSUM)
            )
            tpsum = ctx.enter_context(
                tc.tile_pool(name="tps", bufs=2, space=bass.MemorySpace.PSUM)
            )

            w5sb = const.tile([128, KC, G5], F32, tag="w5")
            nc.sync.dma_start(w5sb[:], w5_d[:])
            w6sb = const.tile([128, KC, H], F32, tag="w6")
            nc.sync.dma_start(w6sb[:], w6_d[:])
            b5sb = const.tile([1, G5], F32, tag="b5")
            nc.sync.dma_start(b5sb[:], b5_d[:])
            b6sb = const.tile([1, H], F32, tag="b6")
            nc.sync.dma_start(b6sb[:], b6_d[:])
            idsb = const.tile([bsh, bsh], F32, tag="id")
            nc.sync.dma_start(idsb[:], id_d[:])
            masksb = const.tile([bsh, t_steps], F32, tag="mask")
            nc.sync.dma_start(masksb[:], mask_d[:])
            ones32 = const.tile([1, 32], F32, tag="ones32")
            nc.vector.memset(ones32[:], 1.0)
            ones16 = const.tile([1, bsh], F32, tag="ones16")
            nc.vector.memset(ones16[:], 1.0)

            ht = hpool.tile([bsh, H], F32, tag="h")
            nc.vector.memset(ht[:], 0.0)
            # mc tile: m_t at partitions 0:16 (written each step by ACT),
            # c_{t-1} at partitions 32:48 (persistent state)
            mct = mcpool.tile([48, H], F32, tag="mc")
            nc.vector.memset(mct[32:48, :], 0.0)

            for t in range(t_steps):
                xt = xpool.tile([128, KC, bsh], F32, tag="xt")
                nc.sync.dma_start(
                    xt[:], xT_d[t].rearrange("(k p) b -> p k b", p=128)
                )

                # u^T = h^T + x^T
                ptr = tpsum.tile([128, KC * bsh], F32, tag="ptr")
                for k in range(KC):
                    nc.tensor.transpose(
                        ptr[:, k * bsh : (k + 1) * bsh],
                        ht[:, k * 128 : (k + 1) * 128],
                        idsb[:],
                    )
                uT = upool.tile([128, KC, bsh], F32, tag="uT")
                for k in range(KC):
                    nc.vector.scalar_tensor_tensor(
                        uT[:, k, :],
                        ptr[:, k * bsh : (k + 1) * bsh],
                        1.0,
                        xt[:, k, :],
                        op0=ALU.mult,
                        op1=ALU.add,
                    )

                # wave2 first (depends only on xt): m uses uT though; px5 only xt
                gB = gpsB.tile([64, H], F32, tag="gB")
                nc.tensor.matmul(
                    gB[32:64, :], ones32[:], b6sb[:],
                    start=True, stop=False, tile_position=(0, 32), skip_group_check=True,
                )
                for k in range(KC):
                    nc.tensor.matmul(
                        gB[32:48, :], xt[:, k, :], w6sb[:, k, :],
                        start=False, stop=(k == KC - 1), tile_position=(0, 32), skip_group_check=True,
                    )

                # wave1: gates i,f,o,hw in 4 col groups of one bank
                gA = gpsA.tile([128, H], F32, tag="gA")
                for n in range(4):
                    reg16 = gA[32 * n : 32 * n + bsh, :]
                    wslice = w5sb[:, :, n * 512 : (n + 1) * 512]
                    nc.tensor.matmul(
                        gA[32 * n : 32 * n + 32, :], ones32[:],
                        b5sb[:, n * 512 : (n + 1) * 512],
                        start=True, stop=False, tile_position=(0, 32 * n), skip_group_check=True,
                    )
                    for k in range(KC):
                        nc.tensor.matmul(
                            reg16, uT[:, k, :], wslice[:, k, :],
                            start=False, stop=(k == KC - 1),
                            tile_position=(0, 32 * n), skip_group_check=True,
                        )
                # m gate into gB group 0
                nc.tensor.matmul(
                    gB[0:32, :], ones32[:], b5sb[:, 4 * 512 : 5 * 512],
                    start=True, stop=False, tile_position=(0, 0), skip_group_check=True,
                )
                for k in range(KC):
                    nc.tensor.matmul(
                        gB[0:bsh, :], uT[:, k, :],
                        w5sb[:, k, 4 * 512 : 5 * 512],
                        start=False, stop=(k == KC - 1), tile_position=(0, 0), skip_group_check=True,
                    )

                # activations: one sigmoid over [112,512] covers i,f,o,hw
                gsig = spool.tile([128, H], F32, tag="gsig")
                nc.scalar.activation(gsig[0:112, :], gA[0:112, :], AF.Sigmoid)
                mcn = mcpool.tile([48, H], F32, tag="mc")
                nc.scalar.activation(mcn[0:bsh, :], gB[0:bsh, :], AF.Tanh)
                px5 = spool.tile([bsh, H], F32, tag="px5")
                nc.scalar.copy(px5[:], gB[32:48, :])

                # [im; fc] = [sig_i; sig_f] * [m; c_prev]   (one packed op)
                # needs c_prev at mcn[32:48]: copy? No — c_prev lives in mct.
                imfc = spool.tile([48, H], F32, tag="imfc")
                nc.vector.tensor_mul(imfc[0:bsh, :], gsig[0:bsh, :], mcn[0:bsh, :])
                nc.vector.tensor_mul(
                    imfc[32:48, :], gsig[32:48, :], mct[32:48, :]
                )
                # c_t -> mcn[32:48]
                nc.vector.tensor_add(
                    mcn[32:48, :], imfc[0:bsh, :], imfc[32:48, :]
                )
                tch = spool.tile([bsh, H], F32, tag="tch")
                nc.scalar.activation(tch[:], mcn[32:48, :], AF.Tanh)
                h1 = spool.tile([bsh, H], F32, tag="h1")
                nc.vector.tensor_mul(h1[:], gsig[64:80, :], tch[:])
                d = spool.tile([bsh, H], F32, tag="d")
                nc.vector.tensor_sub(d[:], h1[:], px5[:])
                e = spool.tile([bsh, H], F32, tag="e")
                nc.vector.tensor_mul(e[:], gsig[96:112, :], d[:])
                hn = spool.tile([bsh, H], F32, tag="hn")
                nc.vector.tensor_add(hn[:], e[:], px5[:])
                hf = hpool.tile([bsh, H], F32, tag="h")
                nc.vector.tensor_scalar_mul(hf[:], hn[:], masksb[:, t : t + 1])

                nc.sync.dma_start(out_d[:, t, :], hf[:])

                ht = hf
                mct = mcn

    nc.compile()
    return nc


def build_nc_v3(t_steps=T, bsh=BSH):
    """v1 structure with: float32r matmul operands (1 cyc/row vs fp32's 4),
    contiguous x^T DMA layout [T,128,KC,bsh], bf16 gate/h-path elementwise
    (c stays fp32), px5 evacuated to bf16 via ACT, output stores on SWDGE."""
    F32R = mybir.dt.float32r
    BF16 = mybir.dt.bfloat16
    nc = bacc.Bacc(
        "TRN2",
        target_bir_lowering=False,
        debug=False,
        enable_asserts=False,
        num_devices=NCORES,
    )
    xT_d = nc.dram_tensor("xT", [t_steps, 128, KC, bsh], F32R, kind="ExternalInput")
    mask_d = nc.dram_tensor("maskT", [bsh, t_steps], F32, kind="ExternalInput")
    w5_d = nc.dram_tensor("w5", [128, KC, G5], F32R, kind="ExternalInput")
    w6_d = nc.dram_tensor("w6", [128, KC, H], F32R, kind="ExternalInput")
    b5_d = nc.dram_tensor("b5", [1, G5], F32R, kind="ExternalInput")
    b6_d = nc.dram_tensor("b6", [1, H], F32R, kind="ExternalInput")
    id_d = nc.dram_tensor("ident", [bsh, bsh], F32R, kind="ExternalInput")
    ones_d = nc.dram_tensor("onesv", [1, bsh], F32R, kind="ExternalInput")
    h0_d = nc.dram_tensor("h0", [bsh, H], F32R, kind="ExternalInput")
    out_d = nc.dram_tensor("out", [bsh, t_steps, H], F32, kind="ExternalOutput")

    with tile.TileContext(nc) as tc:
        with ExitStack() as ctx:
            const = ctx.enter_context(tc.tile_pool(name="const", bufs=1))
            xpool = ctx.enter_context(tc.tile_pool(name="xp", bufs=4))
            upool = ctx.enter_context(tc.tile_pool(name="up", bufs=2))
            hpool = ctx.enter_context(tc.tile_pool(name="hp", bufs=2))
            cpool = ctx.enter_context(tc.tile_pool(name="cp", bufs=2))
            spool = ctx.enter_context(tc.tile_pool(name="sp", bufs=2))
            gpsum = ctx.enter_context(
                tc.tile_pool(name="gps", bufs=1, space=bass.MemorySpace.PSUM)
            )
            ppsum = ctx.enter_context(
                tc.tile_pool(name="pps", bufs=2, space=bass.MemorySpace.PSUM)
            )
            tpsum = ctx.enter_context(
                tc.tile_pool(name="tps", bufs=1, space=bass.MemorySpace.PSUM)
            )

            w5sb = const.tile([128, KC, G5], F32R, tag="w5")
            nc.sync.dma_start(w5sb[:], w5_d[:])
            w6sb = const.tile([128, KC, H], F32R, tag="w6")
            nc.sync.dma_start(w6sb[:], w6_d[:])
            b5sb = const.tile([1, G5], F32R, tag="b5")
            nc.sync.dma_start(b5sb[:], b5_d[:])
            b6sb = const.tile([1, H], F32R, tag="b6")
            nc.sync.dma_start(b6sb[:], b6_d[:])
            idsb = const.tile([bsh, bsh], F32R, tag="id")
            nc.sync.dma_start(idsb[:], id_d[:])
            masksb = const.tile([bsh, t_steps], F32, tag="mask")
            nc.sync.dma_start(masksb[:], mask_d[:])
            ones1 = const.tile([1, bsh], F32R, tag="ones")
            nc.sync.dma_start(ones1[:], ones_d[:])

            ht = hpool.tile([bsh, H], F32R, tag="h")
            nc.sync.dma_start(ht[:], h0_d[:])
            ct = cpool.tile([bsh, H], F32, tag="c")
            nc.vector.memset(ct[:], 0.0)

            for t in range(t_steps):
                xt = xpool.tile([128, KC, bsh], F32R, tag="xt")
                nc.sync.dma_start(xt[:], xT_d[t])

                ptr = tpsum.tile([128, KC * bsh], F32R, tag="ptr")
                for k in range(KC):
                    nc.tensor.transpose(
                        ptr[:, k * bsh : (k + 1) * bsh],
                        ht[:, k * 128 : (k + 1) * 128],
                        idsb[:],
                    )
                uT = upool.tile([128, KC, bsh], F32R, tag="uT")
                for k in range(KC):
                    nc.vector.scalar_tensor_tensor(
                        uT[:, k, :],
                        ptr[:, k * bsh : (k + 1) * bsh],
                        1.0,
                        xt[:, k, :],
                        op0=ALU.mult,
                        op1=ALU.add,
                    )

                g5 = gpsum.tile([bsh, G5], F32, tag="g5")
                for n in range(5):
                    gb = g5[:, n * 512 : (n + 1) * 512]
                    for k in range(KC):
                        nc.tensor.matmul(
                            gb,
                            uT[:, k, :],
                            w5sb[:, k, n * 512 : (n + 1) * 512],
                            start=(k == 0),
                            stop=False,
                        )
                    nc.tensor.matmul(
                        gb,
                        ones1[:],
                        b5sb[:, n * 512 : (n + 1) * 512],
                        start=False,
                        stop=True,
                    )

                p6 = ppsum.tile([bsh, H], F32, tag="p6")
                for k in range(KC):
                    nc.tensor.matmul(
                        p6[:], xt[:, k, :], w6sb[:, k, :],
                        start=(k == 0), stop=False,
                    )
                nc.tensor.matmul(p6[:], ones1[:], b6sb[:], start=False, stop=True)

                gs = spool.tile([bsh, 4 * H], F32, tag="gs")
                nc.scalar.activation(gs[:], g5[:, 0 : 4 * H], AF.Sigmoid)
                ms = spool.tile([bsh, H], F32, tag="ms")
                nc.scalar.activation(ms[:], g5[:, 4 * H : 5 * H], AF.Tanh)
                px5 = spool.tile([bsh, H], F32, tag="px5")
                nc.scalar.copy(px5[:], p6[:])

                im = spool.tile([bsh, H], F32, tag="im")
                nc.vector.tensor_mul(im[:], gs[:, 0:512], ms[:])
                fc = spool.tile([bsh, H], F32, tag="fc")
                nc.vector.tensor_mul(fc[:], gs[:, 512:1024], ct[:])
                cn = cpool.tile([bsh, H], F32, tag="c")
                nc.vector.tensor_add(cn[:], im[:], fc[:])
                tch = spool.tile([bsh, H], F32, tag="tch")
                nc.scalar.activation(tch[:], cn[:], AF.Tanh)
                h1 = spool.tile([bsh, H], F32, tag="h1")
                nc.vector.tensor_mul(h1[:], gs[:, 1024:1536], tch[:])
                d = spool.tile([bsh, H], F32, tag="d")
                nc.vector.tensor_sub(d[:], h1[:], px5[:])
                e = spool.tile([bsh, H], F32, tag="e")
                nc.vector.tensor_mul(e[:], gs[:, 1536:2048], d[:])
                hn = spool.tile([bsh, H], F32, tag="hn")
                nc.vector.tensor_add(hn[:], e[:], px5[:])
                hf = hpool.tile([bsh, H], F32R, tag="h")
                nc.vector.tensor_scalar_mul(hf[:], hn[:], masksb[:, t : t + 1])

                nc.gpsimd.dma_start(out_d[:, t, :], hf[:])

                ht = hf
                ct = cn

    nc.compile()
    return nc


def build_nc_v4(t_steps=T, bsh=BSH):
    """v3 + 8-step batching: x-loads and h-stores batched (1 DMA / 8 steps),
    px5 computed for 8 steps in one M=128 matmul group (the batched x tile is
    the stationary operand), evacuated by a partition-relocating PSUM->SBUF
    DMA.  Per-step PE drops from 34 to ~24.6 matmuls."""
    F32R = mybir.dt.float32r
    SB = 8  # step block
    assert t_steps % SB == 0
    nc = bacc.Bacc(
        "TRN2",
        target_bir_lowering=False,
        debug=False,
        enable_asserts=False,
        num_devices=NCORES,
    )
    xT_d = nc.dram_tensor(
        "xT", [t_steps // SB, 128, KC, SB * bsh], F32R, kind="ExternalInput"
    )
    mask_d = nc.dram_tensor("maskT", [bsh, t_steps], F32, kind="ExternalInput")
    maskb_d = nc.dram_tensor(
        "maskB", [128, t_steps // SB], F32, kind="ExternalInput"
    )
    w5_d = nc.dram_tensor("w5", [128, KC, G5], F32R, kind="ExternalInput")
    w6_d = nc.dram_tensor("w6", [128, KC, H], F32R, kind="ExternalInput")
    b5_d = nc.dram_tensor("b5", [1, G5], F32R, kind="ExternalInput")
    b6_d = nc.dram_tensor("b6", [1, H], F32R, kind="ExternalInput")
    id_d = nc.dram_tensor("ident", [bsh, bsh], F32R, kind="ExternalInput")
    ones_d = nc.dram_tensor("onesv", [1, 128], F32R, kind="ExternalInput")
    h0_d = nc.dram_tensor("h0", [bsh, H], F32R, kind="ExternalInput")
    out_d = nc.dram_tensor("out", [bsh, t_steps, H], F32, kind="ExternalOutput")

    with tile.TileContext(nc) as tc:
        with ExitStack() as ctx:
            const = ctx.enter_context(tc.tile_pool(name="const", bufs=1))
            xpool = ctx.enter_context(tc.tile_pool(name="xp", bufs=3))
            upool = ctx.enter_context(tc.tile_pool(name="up", bufs=2))
            hpool = ctx.enter_context(tc.tile_pool(name="hp", bufs=2))
            cpool = ctx.enter_context(tc.tile_pool(name="cp", bufs=2))
            spool = ctx.enter_context(tc.tile_pool(name="sp", bufs=2))
            pxpool = ctx.enter_context(tc.tile_pool(name="pxp", bufs=2))
            gpsum = ctx.enter_context(
                tc.tile_pool(name="gps", bufs=1, space=bass.MemorySpace.PSUM)
            )
            ppsum = ctx.enter_context(
                tc.tile_pool(name="pps", bufs=1, space=bass.MemorySpace.PSUM)
            )
            tpsum = ctx.enter_context(
                tc.tile_pool(name="tps", bufs=2, space=bass.MemorySpace.PSUM)
            )

            w5sb = const.tile([128, KC, G5], F32R, tag="w5")
            nc.sync.dma_start(w5sb[:], w5_d[:])
            w6sb = const.tile([128, KC, H], F32R, tag="w6")
            nc.sync.dma_start(w6sb[:], w6_d[:])
            b5sb = const.tile([1, G5], F32R, tag="b5")
            nc.sync.dma_start(b5sb[:], b5_d[:])
            b6sb = const.tile([1, H], F32R, tag="b6")
            nc.sync.dma_start(b6sb[:], b6_d[:])
            idsb = const.tile([bsh, bsh], F32R, tag="id")
            nc.sync.dma_start(idsb[:], id_d[:])
            masksb = const.tile([bsh, t_steps], F32, tag="mask")
            nc.sync.dma_start(masksb[:], mask_d[:])
            maskbsb = const.tile([128, t_steps // SB], F32, tag="maskb")
            nc.sync.dma_start(maskbsb[:], maskb_d[:])
            ones1 = const.tile([1, 128], F32R, tag="ones")
            nc.sync.dma_start(ones1[:], ones_d[:])

            ht = hpool.tile([bsh, H], F32R, tag="h0init")
            nc.sync.dma_start(ht[:], h0_d[:])
            ct = cpool.tile([bsh, H], F32, tag="c")
            nc.vector.memset(ct[:], 0.0)

            for t0 in range(0, t_steps, SB):
                # batched x load for 8 steps: [128, KC, SB*bsh]
                xt8 = xpool.tile([128, KC, SB * bsh], F32R, tag="xt8")
                nc.sync.dma_start(xt8[:], xT_d[t0 // SB])

                # px5 for 8 steps: psum [128(t*16+b), 512]
                p6b = ppsum.tile([128, H], F32, tag="p6b")
                nc.tensor.matmul(
                    p6b[:], ones1[:], b6sb[:], start=True, stop=False,
                    skip_group_check=True,
                )
                for k in range(KC):
                    nc.tensor.matmul(
                        p6b[:], xt8[:, k, :], w6sb[:, k, :],
                        start=False, stop=(k == KC - 1), skip_group_check=True,
                    )
                # evac PSUM -> SBUF (base-preserving), then relocate via DMA
                p6sb = pxpool.tile([128, H], F32, tag="p6sb")
                nc.scalar.copy(p6sb[:], p6b[:])
                p6m = pxpool.tile([128, H], F32, tag="p6m")
                nc.vector.tensor_scalar_mul(
                    p6m[:], p6sb[:], maskbsb[:, t0 // SB : t0 // SB + 1]
                )
                px8m = pxpool.tile([bsh, SB, H], F32, tag="px8m")
                for s in range(SB):
                    nc.sync.dma_start(
                        px8m[:, s, :], p6m[s * bsh : (s + 1) * bsh, :]
                    )

                hstore = hpool.tile([bsh, SB, H], F32R, tag="hst")

                for s in range(SB):
                    t = t0 + s
                    g5 = gpsum.tile([bsh, G5], F32, tag="g5")
                    for n in range(5):
                        nc.tensor.matmul(
                            g5[:, n * 512 : (n + 1) * 512],
                            ones1[:, 0:bsh],
                            b5sb[:, n * 512 : (n + 1) * 512],
                            start=True,
                            stop=False,
                            skip_group_check=True,
                        )
                    ptr = tpsum.tile([128, KC * bsh], F32R, tag="ptr")
                    for k in range(KC):
                        nc.tensor.transpose(
                            ptr[:, k * bsh : (k + 1) * bsh],
                            ht[:, k * 128 : (k + 1) * 128],
                            idsb[:],
                        )
                    uT = upool.tile([128, KC, bsh], F32R, tag="uT")
                    nc.vector.scalar_tensor_tensor(
                        uT[:, :, :],
                        ptr[:].rearrange("p (k b) -> p k b", b=bsh),
                        1.0,
                        xt8[:, :, s * bsh : (s + 1) * bsh],
                        op0=ALU.mult,
                        op1=ALU.add,
                    )
                    for n in (0, 1, 4, 2, 3):  # i, f, m, o, hw
                        gb = g5[:, n * 512 : (n + 1) * 512]
                        for k in range(KC):
                            nc.tensor.matmul(
                                gb,
                                uT[:, k, :],
                                w5sb[:, k, n * 512 : (n + 1) * 512],
                                start=False,
                                stop=(k == KC - 1),
                                skip_group_check=True,
                            )
                        if n == 4:
                            ms = spool.tile([bsh, H], F32, tag="ms")
                            nc.scalar.activation(
                                ms[:], g5[:, 4 * H : 5 * H], AF.Tanh
                            )
                        elif n == 1:
                            gs = spool.tile([bsh, 4 * H], F32, tag="gs")
                            nc.scalar.activation(
                                gs[:, 0 : 2 * H], g5[:, 0 : 2 * H], AF.Sigmoid
                            )
                        elif n == 3:
                            nc.scalar.activation(
                                gs[:, 2 * H : 4 * H], g5[:, 2 * H : 4 * H],
                                AF.Sigmoid,
                            )

                    im = spool.tile([bsh, H], F32, tag="im")
                    nc.vector.tensor_mul(im[:], gs[:, 0:512], ms[:])
                    fc = spool.tile([bsh, H], F32, tag="fc")
                    nc.vector.tensor_mul(fc[:], gs[:, 512:1024], ct[:])
                    cn = cpool.tile([bsh, H], F32, tag="c")
                    nc.vector.tensor_add(cn[:], im[:], fc[:])
                    tch = spool.tile([bsh, H], F32, tag="tch")
                    nc.scalar.activation(tch[:], cn[:], AF.Tanh)
                    hwm = spool.tile([bsh, H], F32, tag="hwm")
                    nc.vector.tensor_scalar_mul(
                        hwm[:], gs[:, 1536:2048], masksb[:, t : t + 1]
                    )
                    h1 = spool.tile([bsh, H], F32, tag="h1")
                    nc.vector.tensor_mul(h1[:], gs[:, 1024:1536], tch[:])
                    d = spool.tile([bsh, H], F32, tag="d")
                    e = spool.tile([bsh, H], F32, tag="e")
                    hf = hstore[:, s, :]
                    for hh in range(2):
                        cs = slice(hh * 256, (hh + 1) * 256)
                        nc.vector.tensor_sub(
                            d[:, cs], h1[:, cs], px8m[:, s, cs]
                        )
                        nc.vector.tensor_mul(e[:, cs], hwm[:, cs], d[:, cs])
                        nc.vector.tensor_add(
                            hf[:, cs], e[:, cs], px8m[:, s, cs]
                        )

                    ht = hf
                    ct = cn

                nc.gpsimd.dma_start(out_d[:, t0 : t0 + SB, :], hstore[:])

    nc.compile()
    return nc


def _prep_shared(W_in, b_in):
    cols5 = np.r_[0:1024, 1536:2560, 1024:1536]  # i, f, o, hw, m
    W5 = np.ascontiguousarray(W_in[:, cols5], np.float32)
    b5 = (2.0 * b_in[cols5]).astype(np.float32)[None, :]
    W6 = np.ascontiguousarray(W_in[:, 2560:3072], np.float32)
    b6 = b_in[2560:3072].astype(np.float32)[None, :]
    # [H, N] -> [128, KC, N] with h = k*128 + p
    w5r = np.ascontiguousarray(W5.reshape(KC, 128, G5).transpose(1, 0, 2))
    w6r = np.ascontiguousarray(W6.reshape(KC, 128, H).transpose(1, 0, 2))
    return w5r, b5, w6r, b6


import os

VARIANT = int(os.environ.get("LSTM_KERNEL_VARIANT", "4"))


def kernel(x, lengths, W_in, b_in):
    x = np.asarray(x, np.float32)
    lengths = np.asarray(lengths).astype(np.int64)
    W_in = np.asarray(W_in, np.float32)
    b_in = np.asarray(b_in, np.float32)

    w5r, b5, w6r, b6 = _prep_shared(W_in, b_in)
    ident = np.eye(BSH, dtype=np.float32)
    mask = (np.arange(T)[None, :] < lengths[:, None]).astype(np.float32)

    nc = build_nc(variant=VARIANT)

    in_maps = []
    for j in range(NCORES):
        rows = slice(BSH * j, BSH * (j + 1))
        xT = np.ascontiguousarray(x[rows].transpose(1, 2, 0))  # [T, H, bsh]
        if VARIANT == 3:
            xT = np.ascontiguousarray(
                xT.reshape(T, KC, 128, BSH).transpose(0, 2, 1, 3)
            )  # [T, 128, KC, bsh]
        elif VARIANT >= 4:
            xT = np.ascontiguousarray(
                xT.reshape(T // 8, 8, KC, 128, BSH)
                .transpose(0, 3, 2, 1, 4)
                .reshape(T // 8, 128, KC, 8 * BSH)
            )  # [T/8, 128, KC, 8*bsh]
        m = {
            "xT": xT,
            "maskT": np.ascontiguousarray(mask[rows]),
            "w5": w5r,
            "w6": w6r,
            "b5": b5,
            "b6": b6,
            "ident": ident,
        }
        if VARIANT >= 3:
            m["onesv"] = np.ones((1, BSH if VARIANT == 3 else 128), np.float32)
            m["h0"] = np.zeros((BSH, H), np.float32)
        if VARIANT >= 4:
            mb = mask[rows].T.reshape(T // 8, 128).T
            m["maskB"] = np.ascontiguousarray(mb.astype(np.float32))
        in_maps.append(m)

    trace = bool(int(os.environ.get("LSTM_TRACE", "0")))
    res = run_bass_kernel_spmd(nc, in_maps, list(range(NCORES)), trace=trace)
    if res.exec_time_ns is not None:
        print(f"HW exec time: {res.exec_time_ns} ns", flush=True)
    if trace and res.profile_json is not None:
        import json

        with open("/tmp/lstm_profile.json", "w") as f:
            json.dump(res.profile_json, f)
        print("profile saved to /tmp/lstm_profile.json", flush=True)
    out = np.concatenate([res.results[j]["out"] for j in range(NCORES)], axis=0)
    return out.astype(np.float32)



# revision 3
# speedup vs baseline: 4.8607x; 4.8607x over previous
"""AugmentedLstm TRN2 kernel v8: hidden-major, three-source gate matmuls.

g(t) = (x_t + h_{t-1}) @ W + 2b is accumulated in PSUM from three moving
sources instead of materializing u = x + h:
    bias + x_t @ W      -- x-only, schedulable arbitrarily early
    p1 @ W (if-chunks)  -- p1 = (1-hw)*px5, ready mid-step
    e2 @ W (if-chunks)  -- e2 = (o*hw)*tanh(c), the last value produced;
                           only the 8 if-chunks gate sig-if, so the
                           critical tail is e2 -> 32 matmuls -> sig-if.
m/o/hw/nhw chunks take h = e2 + p1 (one off-path DVE op; h also feeds the
output store) instead of separate p1/e2 passes, halving their matmuls.

Per step: PE ~244 matmuls (x:96+bias3, p1-if:32, e2-if:32, h-mrest:64,
px:17), ACT 4 ops, DVE 6 ops (imfc, cn, p1, ohw, e2, h).
"""

import numpy as np
from contextlib import ExitStack

import concourse.bass as bass
import concourse.bacc as bacc
import concourse.tile as tile
import concourse.mybir as mybir

F32 = mybir.dt.float32
BF16 = mybir.dt.bfloat16
AF = mybir.ActivationFunctionType
ALU = mybir.AluOpType

B, T, H = 128, 512, 512
NCORES = 8
BSH = B // NCORES
KC = H // 128
NCH = 24                   # i0-3 f0-3 o0-3 hw0-3 nhw0-3 m0-3
SB = 8


def build_nc_v8(t_steps=T, bsh=BSH):
    nc = bacc.Bacc(
        "TRN2",
        target_bir_lowering=False,
        debug=False,
        enable_asserts=False,
        num_devices=NCORES,
    )
    nblk = t_steps // SB
    xT_d = nc.dram_tensor("xT", [nblk, 128, KC, SB, bsh], BF16, kind="ExternalInput")
    w5_d = nc.dram_tensor("w5", [128, KC, NCH, 128], BF16, kind="ExternalInput")
    w6_d = nc.dram_tensor("w6", [128, KC, KC, 128], BF16, kind="ExternalInput")
    b5T_d = nc.dram_tensor("b5T", [NCH, 128], BF16, kind="ExternalInput")
    ind5_d = nc.dram_tensor("ind5", [NCH, NCH * bsh], BF16, kind="ExternalInput")
    b6T_d = nc.dram_tensor("b6T", [KC, 128], BF16, kind="ExternalInput")
    ind6_d = nc.dram_tensor("ind6", [KC, KC * bsh], BF16, kind="ExternalInput")
    out_d = nc.dram_tensor("out", [nblk, 128, SB, KC, bsh], BF16, kind="ExternalOutput")

    with tile.TileContext(nc) as tc:
        with ExitStack() as ctx:
            const = ctx.enter_context(tc.tile_pool(name="const", bufs=1))
            xpool = ctx.enter_context(tc.tile_pool(name="xp", bufs=3))
            mcpool = ctx.enter_context(tc.tile_pool(name="mcp", bufs=2))
            spool = ctx.enter_context(tc.tile_pool(name="sp", bufs=2))
            pxpool = ctx.enter_context(tc.tile_pool(name="pxp", bufs=2))
            hpool = ctx.enter_context(tc.tile_pool(name="hp", bufs=2))
            gifp = ctx.enter_context(
                tc.tile_pool(name="gif", bufs=2, space=bass.MemorySpace.PSUM))
            gmp = ctx.enter_context(
                tc.tile_pool(name="gm", bufs=2, space=bass.MemorySpace.PSUM))
            grp = ctx.enter_context(
                tc.tile_pool(name="gr", bufs=2, space=bass.MemorySpace.PSUM))
            ppsum = ctx.enter_context(
                tc.tile_pool(name="pps", bufs=2, space=bass.MemorySpace.PSUM))

            w5sb = const.tile([128, KC, NCH, 128], BF16, tag="w5")
            nc.sync.dma_start(w5sb[:], w5_d[:])
            w6sb = const.tile([128, KC, KC, 128], BF16, tag="w6")
            nc.sync.dma_start(w6sb[:], w6_d[:])
            b5if = const.tile([8, 128], BF16, tag="b5if")
            nc.sync.dma_start(b5if[:], b5T_d[0:8])
            b5m = const.tile([4, 128], BF16, tag="b5m")
            nc.sync.dma_start(b5m[:], b5T_d[20:24])
            b5r = const.tile([12, 128], BF16, tag="b5r")
            nc.sync.dma_start(b5r[:], b5T_d[8:20])
            indif = const.tile([8, 8 * bsh], BF16, tag="indif")
            nc.sync.dma_start(indif[:], ind5_d[0:8, 0 : 8 * bsh])
            indm = const.tile([4, 4 * bsh], BF16, tag="indm")
            nc.sync.dma_start(indm[:], ind5_d[20:24, 20 * bsh : 24 * bsh])
            indr = const.tile([12, 12 * bsh], BF16, tag="indr")
            nc.sync.dma_start(indr[:], ind5_d[8:20, 8 * bsh : 20 * bsh])
            b6Tsb = const.tile([KC, 128], BF16, tag="b6T")
            nc.sync.dma_start(b6Tsb[:], b6T_d[:])
            ind6sb = const.tile([KC, KC * bsh], BF16, tag="ind6")
            nc.sync.dma_start(ind6sb[:], ind6_d[:])

            # m (bf16, ACT-written each step) and c (f32 state) tiles
            mtt = mcpool.tile([128, KC, bsh], BF16, tag="mt")
            ct = mcpool.tile([128, KC, bsh], F32, tag="ct")
            nc.vector.memset(ct[:], 0.0)

            def mm4(dst, ch, src_t, stop):
                for ck in range(KC):
                    nc.tensor.matmul(
                        dst, w5sb[:, ck, ch, :], src_t[:, ck, :],
                        start=False, stop=stop and (ck == KC - 1),
                        skip_group_check=True,
                    )

            def emit_gates_x(xt, s):
                """bias + x-part of g(t): allocs tiles, no recurrent deps."""
                gif = gifp.tile([128, 8, bsh], F32, tag="gif", name="gif")
                gm = gmp.tile([128, KC, bsh], F32, tag="gm", name="gm")
                gr = grp.tile([128, 12, bsh], F32, tag="gr", name="gr")
                xs = xt[:, :, s, :]
                nc.tensor.matmul(
                    gif[:].rearrange("p c b -> p (c b)"), b5if[:], indif[:],
                    start=True, stop=False, skip_group_check=True)
                for ch in range(8):
                    mm4(gif[:, ch, :], ch, xs, False)
                nc.tensor.matmul(
                    gm[:].rearrange("p c b -> p (c b)"), b5m[:], indm[:],
                    start=True, stop=False, skip_group_check=True)
                for ch in range(20, 24):
                    mm4(gm[:, ch - 20, :], ch, xs, False)
                nc.tensor.matmul(
                    gr[:].rearrange("p c b -> p (c b)"), b5r[:], indr[:],
                    start=True, stop=False, skip_group_check=True)
                for ch in range(8, 20):
                    mm4(gr[:, ch - 8, :], ch, xs, False)
                return gif, gm, gr

            def emit_gates_p1(g3, p1_t):
                gif, gm, gr = g3
                for ch in range(8):
                    mm4(gif[:, ch, :], ch, p1_t, False)

            def emit_gates_e2(g3, e2_t, h_t):
                gif, gm, gr = g3
                for ch in range(8):
                    mm4(gif[:, ch, :], ch, e2_t, True)
                for ch in range(20, 24):
                    mm4(gm[:, ch - 20, :], ch, h_t, True)
                for ch in range(8, 20):
                    mm4(gr[:, ch - 8, :], ch, h_t, True)

            def emit_px(xt, s):
                px = ppsum.tile([128, KC, bsh], F32, tag="px", name="px")
                nc.tensor.matmul(
                    px[:].rearrange("p c b -> p (c b)"),
                    b6Tsb[:], ind6sb[:],
                    start=True, stop=False, skip_group_check=True)
                for cc in range(KC):
                    for ck in range(KC):
                        nc.tensor.matmul(
                            px[:, cc, :], w6sb[:, ck, cc, :], xt[:, ck, s, :],
                            start=False, stop=(ck == KC - 1),
                            skip_group_check=True)
                return px

            nblk_ = nblk
            xtiles = {}
            for k in range(min(2, nblk_)):
                xtiles[k] = xpool.tile([128, KC, SB, bsh], BF16, tag="xt8",
                                       name=f"xt8_{k}")
                nc.sync.dma_start(xtiles[k][:], xT_d[k])

            # step 0: g(0) = x0 @ W + 2b only (h(-1) = 0); close the groups
            gcur = emit_gates_x(xtiles[0], 0)
            zt = const.tile([128, KC, bsh], BF16, tag="zt")
            nc.vector.memset(zt[:], 0.0)
            emit_gates_p1(gcur, zt)
            emit_gates_e2(gcur, zt, zt)
            px = emit_px(xtiles[0], 0)

            for blk in range(nblk_):
                xt8 = xtiles[blk]
                xt8_next = xtiles.get(blk + 1)
                if blk + 2 < nblk_:
                    xtiles[blk + 2] = xpool.tile(
                        [128, KC, SB, bsh], BF16, tag="xt8",
                        name=f"xt8_{blk + 2}")
                    nc.sync.dma_start(xtiles[blk + 2][:], xT_d[blk + 2])

                hstH = hpool.tile([128, SB, KC, bsh], BF16, tag="hstH")

                for s in range(SB):
                    t = blk * SB + s
                    gif, gm, gr = gcur

                    sigif = spool.tile([128, 8, bsh], BF16, tag="sigif")
                    nc.scalar.activation(sigif[:], gif[:], AF.Sigmoid)
                    mtt = mcpool.tile([128, KC, bsh], BF16, tag="mt")
                    nc.scalar.activation(mtt[:], gm[:], AF.Tanh)
                    sr = spool.tile([128, 12, bsh], BF16, tag="sr")
                    nc.scalar.activation(sr[:], gr[:], AF.Sigmoid)

                    # c path: fc first (needs only sig-f), im waits tanh-m;
                    # im/e2 all-bf16 SBUF for the DVE 2x mode, c stays f32
                    fc = spool.tile([128, KC, bsh], F32, tag="fc")
                    nc.vector.tensor_mul(fc[:], sigif[:, 4:8, :], ct[:])
                    im = spool.tile([128, KC, bsh], BF16, tag="im")
                    nc.vector.tensor_mul(im[:], sigif[:, 0:4, :], mtt[:])
                    cn = mcpool.tile([128, KC, bsh], F32, tag="ct")
                    nc.vector.tensor_add(cn[:], im[:], fc[:])
                    tc_ = spool.tile([128, KC, bsh], BF16, tag="tc")
                    nc.scalar.activation(tc_[:], cn[:], AF.Tanh)

                    # next-step tiles + x-part + px: x-only, fills PE idle
                    if t + 1 < t_steps:
                        if s + 1 < SB:
                            gnext = emit_gates_x(xt8, s + 1)
                            pxn = emit_px(xt8, s + 1)
                        else:
                            gnext = emit_gates_x(xt8_next, 0)
                            pxn = emit_px(xt8_next, 0)
                    else:
                        gnext = None
                        pxn = None

                    # p1 then its matmuls (run inside the tanh-c window)
                    p1 = pxpool.tile([128, KC, bsh], BF16, tag="p1")
                    nc.vector.tensor_mul(p1[:], sr[:, 8:12, :], px[:])
                    if gnext is not None:
                        emit_gates_p1(gnext, p1)
                    ohw = spool.tile([128, KC, bsh], BF16, tag="ohw")
                    nc.vector.tensor_mul(ohw[:], sr[:, 0:4, :], sr[:, 4:8, :])

                    # critical tail
                    e2 = spool.tile([128, KC, bsh], BF16, tag="e2")
                    nc.vector.tensor_mul(e2[:], ohw[:], tc_[:])
                    # h = e2 + p1 (off-path; feeds m/rest matmuls + store)
                    h = hstH[:, s]
                    nc.vector.scalar_tensor_tensor(
                        h, e2[:], 1.0, p1[:], op0=ALU.mult, op1=ALU.add)
                    if gnext is not None:
                        emit_gates_e2(gnext, e2, h)
                        gcur = gnext
                        px = pxn

                    ct = cn

                nc.sync.dma_start(out_d[blk], hstH[:])
                xtiles.pop(blk)

    nc.compile()
    return nc


def prep_inputs_v8(x, lengths, W_in, b_in, t_steps=T):
    iw = np.r_[0:512]
    fw = np.r_[512:1024]
    mw = np.r_[1024:1536]
    ow = np.r_[1536:2048]
    hww = np.r_[2048:2560]
    W5 = np.concatenate([
        W_in[:, iw], W_in[:, fw], W_in[:, ow], W_in[:, hww],
        -W_in[:, hww], W_in[:, mw]], axis=1).astype(np.float32)
    b5 = 2.0 * np.concatenate([
        b_in[iw], b_in[fw], b_in[ow], b_in[hww],
        -b_in[hww], b_in[mw]]).astype(np.float32)
    W6 = W_in[:, 2560:3072].astype(np.float32)
    b6 = b_in[2560:3072].astype(np.float32)

    def bf(a):
        import ml_dtypes
        return a.astype(ml_dtypes.bfloat16)

    w5r = np.ascontiguousarray(W5.reshape(KC, 128, NCH, 128).transpose(1, 0, 2, 3))
    w6r = np.ascontiguousarray(W6.reshape(KC, 128, KC, 128).transpose(1, 0, 2, 3))
    b5T = np.ascontiguousarray(b5.reshape(NCH, 128))
    ind5 = np.zeros((NCH, NCH, BSH), np.float32)
    for ch in range(NCH):
        ind5[ch, ch, :] = 1.0
    ind5 = ind5.reshape(NCH, NCH * BSH)
    b6T = np.ascontiguousarray(b6.reshape(KC, 128))
    ind6 = np.zeros((KC, KC, BSH), np.float32)
    for cc in range(KC):
        ind6[cc, cc, :] = 1.0
    ind6 = ind6.reshape(KC, KC * BSH)

    in_maps = []
    for j in range(NCORES):
        rows = slice(BSH * j, BSH * (j + 1))
        xc = x[rows, :t_steps]
        xT = np.ascontiguousarray(
            xc.reshape(BSH, t_steps // SB, SB, KC, 128).transpose(1, 4, 3, 2, 0))
        in_maps.append({
            "xT": bf(xT), "w5": bf(w5r), "w6": bf(w6r), "b5T": bf(b5T),
            "ind5": bf(ind5), "b6T": bf(b6T), "ind6": bf(ind6),
        })
    return in_maps


def postprocess_v8(results, lengths, t_steps=T):
    mask = (np.arange(t_steps)[None, :] < np.asarray(lengths)[:, None])
    out = np.empty((B, t_steps, H), np.float32)
    for j in range(NCORES):
        rows = slice(BSH * j, BSH * (j + 1))
        h = np.asarray(results[j]["out"], dtype=np.float32)
        out[rows] = h.transpose(4, 0, 2, 3, 1).reshape(BSH, t_steps, H)
    out *= mask[:, :, None]
    return out


def build_nc(t_steps=T, variant=8):
    return build_nc_v8(t_steps)


def kernel(x, lengths, W_in, b_in):
    """Full-input entry point: shards batch over 8 cores, runs the Bass
    kernel SPMD, reassembles the full [B, T, H] float32 output."""
    from concourse.bass_utils import run_bass_kernel_spmd

    x = np.asarray(x, np.float32)
    lengths = np.asarray(lengths).astype(np.int64)
    W_in = np.asarray(W_in, np.float32)
    b_in = np.asarray(b_in, np.float32)

    nc = build_nc_v8(T)
    in_maps = prep_inputs_v8(x, lengths, W_in, b_in, t_steps=T)
    res = run_bass_kernel_spmd(nc, in_maps, list(range(NCORES)))
    if getattr(res, "exec_time_ns", None) is not None:
        print(f"HW exec time: {res.exec_time_ns} ns", flush=True)
    out = postprocess_v8(res.results, lengths, t_steps=T)
    return out.astype(np.float32)
